# revision 72
# baseline (speedup 1.0000x reference)
"""Trainium2 Bass kernel for nn_DimVariationalEmcoder (GCN + 2x TransformerConv VAE encoder).

Strategy (8 NeuronCores, SPMD):
  - Nodes sharded contiguously: core c owns global nodes [c*6250, (c+1)*6250),
    padded to 6272 = 49*128 local rows. Edges partitioned by destination core.
  - Per core, destination nodes are bin-packed into 49 blocks of 128 nodes with
    balanced in-edge counts; edges are grouped by block, sorted into "lo"
    (src table row < 32768) and "hi" tiles so dma_gather's int16 indices reach
    the whole 50176-row table via a rebased view.
  - Phase 1a: h' = (x @ W_gcn) * dinv for own nodes (bf16); AllGather the h'
    table so edge gathers are local.
  - Phase 1b: per 7-block group, batched dma_gather of h'[src] rows; per block,
    aggregate with a streamed fp8 one-hot (slot) matmul into PSUM, add
    self-loop term, * dinv + bias, LeakyReLU -> h2 chunk; transpose (PE) into
    a feature-major tile h2t_sb.
  - Phase 2a: own-stripe kv rows ([k_mu|k_ls|v interleaved], bf16), local q
    and skip tables from h2t_sb; AllGather the kv stripe (the halo exchange).
  - Phase 2b: per 2-block group, batched dma_gather of kv[src]; per block,
    expand q[dst] per edge on the PE (streamed transposed fp8 one-hot x local
    q block), alpha = q.k/8 via DVE mult+reduce, unshifted softmax (exp on
    Act), payload [ex*v | ex], aggregate via the one-hot matmul, normalize,
    add skip, clamp logstd. Pad edges have all-zero one-hot rows/cols, so no
    masking is needed anywhere.
"""

import numpy as np

import concourse.bacc as bacc
import concourse.mybir as mybir
import concourse.tile as tile
from concourse.bass_utils import run_bass_kernel_spmd

# Problem constants (hardcoded per the task contract).
N = 50000
E = 800000
F_IN = 256
H = 128
D = 64
W = 8                    # cores
NLOC = N // W            # 6250
NB = 49                  # blocks per core
NPAD = NB * 128          # 6272 padded local rows
G = W * NPAD             # 50176 global padded table rows
LO_LIM = 32768           # int16 gather index limit
MAX_LOGSTD = 10.0
LEAKY = 0.01
F32 = mybir.dt.float32
BF16 = mybir.dt.bfloat16
FP8 = mybir.dt.float8e4
FP16 = mybir.dt.float16
I16 = mybir.dt.int16
G1B = 7                  # blocks per gather group, phase 1b
G2B = 2                  # blocks per gather group, phase 2b


def configure(n, nb, lo_lim=32768):
    """Reconfigure problem size (for small-scale simulation tests)."""
    global N, NLOC, NB, NPAD, G, LO_LIM
    N = n
    NLOC = n // W
    NB = nb
    NPAD = nb * 128
    G = W * NPAD
    LO_LIM = lo_lim
    assert NLOC <= NPAD


# ----------------------------------------------------------------------------
# Host-side preprocessing
# ----------------------------------------------------------------------------

def _pack_blocks(indeg):
    """Greedy balanced bin-packing of NPAD nodes into NB bins of 128 nodes."""
    import heapq
    order = np.argsort(-indeg, kind="stable")
    bin_cnt = np.zeros(NB, np.int64)
    heap = [(0, b) for b in range(NB)]
    heapq.heapify(heap)
    assign = np.empty(NPAD, np.int64)
    for v in order:
        while True:
            e, b = heapq.heappop(heap)
            if bin_cnt[b] < 128:
                break
        assign[v] = b
        bin_cnt[b] += 1
        heapq.heappush(heap, (e + int(indeg[v]), b))
    perm = np.empty(NPAD, np.int64)
    slot_of = np.empty(NPAD, np.int64)
    fill = np.zeros(NB, np.int64)
    for v in range(NPAD):
        b = assign[v]
        perm[b * 128 + fill[b]] = v
        slot_of[v] = fill[b]
        fill[b] += 1
    return perm, assign, slot_of


def _wrap_idx(a):
    """[L] int array -> [128, L//16] int16 wrapped layout (replicated x8)."""
    w = np.ascontiguousarray(a.reshape(-1, 16).T.astype(np.int16))
    return np.tile(w, (8, 1))


def preprocess(x, edge_index):
    import ml_dtypes
    fp8 = ml_dtypes.float8_e4m3

    src = np.asarray(edge_index[0], dtype=np.int64)
    dst = np.asarray(edge_index[1], dtype=np.int64)
    x = np.asarray(x, dtype=np.float32)

    deg = np.bincount(dst, minlength=N).astype(np.float64) + 1.0
    dinv = (1.0 / np.sqrt(deg)).astype(np.float32)

    # Per-core permutations (destination-side bin packing).
    perms = []
    for c in range(W):
        mask = (dst // NLOC) == c
        dl = dst[mask] - c * NLOC
        indeg = np.bincount(dl, minlength=NPAD).astype(np.int64)
        p, a, s = _pack_blocks(indeg)
        perms.append(p)

    # Global padded table position of every real node.
    pos_of = np.empty(N, np.int64)
    for c in range(W):
        inv = np.empty(NPAD, np.int64)
        inv[perms[c]] = np.arange(NPAD)
        pos_of[c * NLOC:(c + 1) * NLOC] = c * NPAD + inv[:NLOC]

    src_pos = pos_of[src]

    # First pass: per-(core, block) lo/hi counts to size the tile grid.
    per_core = []
    t_lo_max, t_hi_max = 0, 0
    for c in range(W):
        mask = (dst // NLOC) == c
        e_src_pos = src_pos[mask]
        e_dl = dst[mask] - c * NLOC
        e_nl = np.empty(NPAD, np.int64)
        e_nl[perms[c]] = np.arange(NPAD)
        new_local = e_nl[e_dl]
        blk = new_local // 128
        slo = new_local % 128
        is_hi = e_src_pos >= LO_LIM
        per_core.append((e_src_pos, blk, slo, is_hi))
        for b in range(NB):
            m = blk == b
            nlo = int(np.sum(m & ~is_hi))
            nhi = int(np.sum(m & is_hi))
            t_lo_max = max(t_lo_max, -(-nlo // 128))
            t_hi_max = max(t_hi_max, -(-nhi // 128))

    T_LO, T_HI = max(t_lo_max, 1), max(t_hi_max, 1)
    T = T_LO + T_HI

    in_maps = []
    gperm_cores = []
    for c in range(W):
        e_src_pos, blk, slo, is_hi = per_core[c]
        idx_lo = np.zeros((NB, T_LO * 128), np.int64)
        idx_hi = np.zeros((NB, T_HI * 128), np.int64)
        slot_a = np.full((NB, T * 128), 128, np.int64)  # pad slot -> no onehot
        for b in range(NB):
            m = blk == b
            for hi in (False, True):
                mm = m & (is_hi if hi else ~is_hi)
                k = int(mm.sum())
                if hi:
                    idx_hi[b, :k] = e_src_pos[mm] - LO_LIM
                    off = T_LO * 128
                else:
                    idx_lo[b, :k] = e_src_pos[mm]
                    off = 0
                slot_a[b, off:off + k] = slo[mm]

        # one-hot [NB, 128(e), T*128(slot-major)] and its transpose, fp8
        sa = slot_a.reshape(NB, T, 128)
        rng = np.arange(128)
        oh_np = (sa.transpose(0, 2, 1)[:, :, :, None] == rng[None, None, None, :])
        oh_np = np.ascontiguousarray(
            oh_np.reshape(NB, 128, T * 128)).astype(fp8)
        oht_np = (sa[:, None, :, :] == rng[None, :, None, None])
        oht_np = np.ascontiguousarray(
            oht_np.reshape(NB, 128, T * 128)).astype(fp8)

        # per-core x slice in permuted order, chunk-transposed, bf16
        gsel = np.where(perms[c] < NLOC, c * NLOC + perms[c], -1)
        gperm_cores.append(gsel)
        x_own = np.zeros((NPAD, F_IN), np.float32)
        vv = gsel >= 0
        x_own[vv] = x[gsel[vv]]
        x_ownT = np.ascontiguousarray(
            x_own.reshape(NB, 128, F_IN).transpose(0, 2, 1)).astype(
                ml_dtypes.bfloat16)

        dv = np.zeros(NPAD, np.float32)
        sel_src = np.where(perms[c] < NLOC, c * NLOC + perms[c], 0)
        dv[vv] = dinv[sel_src[vv]]
        dinvn_sb = np.ascontiguousarray(dv.reshape(NB, 128).T)

        lane = np.ascontiguousarray(
            slot_a.reshape(NB * T, 128).T).astype(ml_dtypes.bfloat16)
        in_maps.append(dict(
            x_ownT=x_ownT,
            dinvn=dinvn_sb,
            idx_lo=_wrap_idx(idx_lo.reshape(-1)),
            idx_hi=_wrap_idx(idx_hi.reshape(-1)),
            eslot=lane,
            ohm=oh_np,
            oht=oht_np,
        ))

    gperm = np.concatenate(gperm_cores)          # [G] global node id or -1
    return in_maps, gperm, T_LO, T_HI


# ----------------------------------------------------------------------------
# Kernel build
# ----------------------------------------------------------------------------

def build_kernel(T_LO, T_HI, weights):
    T = T_LO + T_HI
    HI0 = LO_LIM if G > LO_LIM else 0  # hi-gather rebase offset
    ndev = 1 if TIMING_1CORE else W
    nc = bacc.Bacc("TRN2", target_bir_lowering=False, debug=False,
                   num_devices=ndev)

    # inputs
    x_ownT = nc.dram_tensor("x_ownT", [NB, F_IN, 128], BF16, kind="ExternalInput")
    dinvn_d = nc.dram_tensor("dinvn", [128, NB], F32, kind="ExternalInput")
    idx_lo_d = nc.dram_tensor("idx_lo", [128, NB * T_LO * 8], I16, kind="ExternalInput")
    idx_hi_d = nc.dram_tensor("idx_hi", [128, NB * T_HI * 8], I16, kind="ExternalInput")
    ohm_d = nc.dram_tensor("ohm", [NB, 128, T * 128], FP8, kind="ExternalInput")
    oht_d = nc.dram_tensor("oht", [NB, 128, T * 128], FP8, kind="ExternalInput")
    eslot_d = nc.dram_tensor("eslot", [128, NB * T], BF16, kind="ExternalInput")
    wnames = ["Wall", "ball", "ident", "iotab"]
    bf16_w = {"Wall", "iotab"}
    w1names = ["Wg0", "Wg1", "bgcn"]
    bf16_w1 = {"Wg0", "Wg1"}
    wshapes = {
        "Wg0": [128, H], "Wg1": [128, H], "bgcn": [128, H],
        "Wall": [H, 512], "ball": [128, 512],
        "ident": [128, 128], "iotab": [128, 128],
    }
    wd = {k: nc.dram_tensor(k, wshapes[k],
                            BF16 if (k in bf16_w or k in bf16_w1) else F32,
                            kind="ExternalInput")
          for k in wnames + w1names}

    out_mu = nc.dram_tensor("out_mu", [NPAD, D], BF16, kind="ExternalOutput")
    out_ls = nc.dram_tensor("out_ls", [NPAD, D], BF16, kind="ExternalOutput")

    # internal DRAM
    hw_bounce = nc.dram_tensor("hw_bounce", [NPAD, H], BF16)
    hw_table = nc.dram_tensor("hw_table", [G, H], BF16, addr_space="Shared")
    kv_bounce = nc.dram_tensor("kv_bounce", [NPAD, 256], BF16)
    kv_table = nc.dram_tensor("kv_table", [G, 256], BF16, addr_space="Shared")

    with tile.TileContext(nc) as tc:
        with (
            tc.tile_pool(name="const", bufs=1) as cp,
            tc.tile_pool(name="persist", bufs=1) as pp,
        ):
            # cross-phase constants
            wt = {}
            for k in wnames:
                t = cp.tile(wshapes[k], BF16 if k in bf16_w else F32, tag=k)
                nc.sync.dma_start(t[:], wd[k][:])
                wt[k] = t
            il = cp.tile([128, NB * T_LO * 8], I16, tag="il")
            nc.sync.dma_start(il[:], idx_lo_d[:])
            ih = cp.tile([128, NB * T_HI * 8], I16, tag="ih")
            nc.sync.dma_start(ih[:], idx_hi_d[:])
            eslot = cp.tile([128, NB * T], BF16, tag="eslot")
            nc.sync.dma_start(eslot[:], eslot_d[:])


            skip_sb = pp.tile([128, NB, 128], BF16, tag="skip")
            qst = pp.tile([128, NB, 128], BF16, tag="qst")

            # ---- Phase 1: GCN ------------------------------------------
            h2tm = tc.tile_pool(name="h2t_pool", bufs=1)
            hp = h2tm.__enter__()
            h2t_sb = hp.tile([128, NPAD], BF16, tag="h2t")
            kvst = hp.tile([128, NB, 256], BF16, tag="kvst")
            p1cm = tc.tile_pool(name="p1const", bufs=1)
            p1cp = p1cm.__enter__()
            for k in w1names:
                t = p1cp.tile(wshapes[k], BF16 if k in bf16_w1 else F32, tag=k)
                nc.sync.dma_start(t[:], wd[k][:])
                wt[k] = t
            dinvn = p1cp.tile([128, NB], F32, tag="dinvn")
            nc.sync.dma_start(dinvn[:], dinvn_d[:])
            hwo = p1cp.tile([128, NB, H], BF16, tag="hwo")

            # ------- Phase 1a: own h' slice, then AllGather the table ----
            with (
                tc.tile_pool(name="p1a_in", bufs=5) as pin,
                tc.tile_pool(name="p1a_ps", bufs=2, space="PSUM") as pps,
            ):
              for _rep in range(REP.get('1a', REPEAT)):
                for g in range(-(-NB // G1B)):
                    b0 = g * G1B
                    nbg = min(G1B, NB - b0)
                    xt = pin.tile([128, nbg, 2, 128], BF16, tag="xt")
                    nc.scalar.dma_start(
                        xt[:], x_ownT[b0:b0 + nbg]
                        .rearrange("g (j p) m -> p g j m", p=128))
                    for j in range(nbg):
                        b = b0 + j
                        ps = pps.tile([128, H], F32, tag="ps")
                        nc.tensor.matmul(ps[:], xt[:, j, 0, :], wt["Wg0"][:],
                                         start=True, stop=False)
                        nc.tensor.matmul(ps[:], xt[:, j, 1, :], wt["Wg1"][:],
                                         start=False, stop=True)
                        # h' = (x @ Wg) * dinv  (norm factored per-node)
                        nc.scalar.mul(hwo[:, b, :], ps[:], dinvn[:, b:b + 1])
                    nc.sync.dma_start(
                        hw_bounce[b0 * 128:(b0 + nbg) * 128, :]
                        .rearrange("(b p) m -> p b m", p=128),
                        hwo[:, b0:b0 + nbg, :])
                    if TIMING_1CORE:
                        nc.sync.dma_start(
                            hw_table[b0 * 128:(b0 + nbg) * 128, :],
                            hw_bounce[b0 * 128:(b0 + nbg) * 128, :])
            if not TIMING_1CORE:
                nc.gpsimd.collective_compute(
                    "AllGather",
                    mybir.AluOpType.bypass,
                    ins=[hw_bounce.ap().opt()],
                    outs=[hw_table.ap().opt()],
                    replica_groups=[list(range(W))],
                )

            # ------- Phase 1b: GCN aggregation + fused kv/q/skip -----------
            NG1 = -(-NB // G1B)
            with (
                tc.tile_pool(name="p1b_g", bufs=3) as pg,
                tc.tile_pool(name="p1b_oh", bufs=4) as poh,
                tc.tile_pool(name="p1b_ps", bufs=3, space="PSUM") as pps1,
                tc.tile_pool(name="p1b_pst", bufs=2, space="PSUM") as ppst,
                tc.tile_pool(name="p1b_pall", bufs=2, space="PSUM") as ppall,
                tc.tile_pool(name="p1b_h2", bufs=4) as ph2,
            ):
              for _rep in range(REP.get('1b', REPEAT)):
                for g in range(NG1):
                    b0 = g * G1B
                    nbg = min(G1B, NB - b0)
                    gl = pg.tile([128, nbg * T_LO, H], BF16, tag="gl")
                    SPL = 2
                    for c0 in range(0, nbg, SPL):
                        cn = min(SPL, nbg - c0)
                        nc.gpsimd.dma_gather(
                            gl[:, c0 * T_LO:(c0 + cn) * T_LO, :],
                            hw_table[:],
                            il[:, (b0 + c0) * T_LO * 8:
                               (b0 + c0 + cn) * T_LO * 8],
                            cn * T_LO * 128, cn * T_LO * 128, H,
                            single_packet=False)
                    gh = pg.tile([128, nbg * T_HI, H], BF16, tag="gh")
                    for c0 in range(0, nbg, SPL):
                        cn = min(SPL, nbg - c0)
                        nc.gpsimd.dma_gather(
                            gh[:, c0 * T_HI:(c0 + cn) * T_HI, :],
                            hw_table[HI0:, :],
                            ih[:, (b0 + c0) * T_HI * 8:
                               (b0 + c0 + cn) * T_HI * 8],
                            cn * T_HI * 128, cn * T_HI * 128, H,
                            single_packet=False)
                    for j in range(nbg):
                        b = b0 + j
                        oh1 = poh.tile([128, T, 128], BF16, tag="oh1")
                        esl = eslot[:, b * T:(b + 1) * T]
                        nc.vector.tensor_tensor(
                            oh1[:],
                            esl.rearrange("p (c one) -> p c one", one=1)
                            .broadcast_to([128, T, 128]),
                            wt["iotab"][:]
                            .rearrange("p (one s) -> p one s", one=1)
                            .broadcast_to([128, T, 128]),
                            op=mybir.AluOpType.is_equal)
                        ps1 = pps1.tile([128, H], F32, tag="ps1")
                        for t in range(T_LO):
                            nc.tensor.matmul(ps1[:], oh1[:, t, :],
                                             gl[:, j * T_LO + t, :],
                                             start=(t == 0), stop=False)
                        for t in range(T_HI):
                            nc.tensor.matmul(ps1[:], oh1[:, T_LO + t, :],
                                             gh[:, j * T_HI + t, :],
                                             start=False, stop=(t == T_HI - 1))
                        # h2 = leaky((agg + h'_own) * dinv + bias)
                        h2c = ph2.tile([128, H], F32, tag="h2c")
                        nc.vector.tensor_tensor(h2c[:], ps1[:], hwo[:, b, :],
                                                op=mybir.AluOpType.add)
                        nc.vector.scalar_tensor_tensor(
                            h2c[:], h2c[:], dinvn[:, b:b + 1], wt["bgcn"][:],
                            op0=mybir.AluOpType.mult, op1=mybir.AluOpType.add)
                        nc.vector.scalar_tensor_tensor(
                            h2c[:], h2c[:], LEAKY, h2c[:],
                            op0=mybir.AluOpType.mult, op1=mybir.AluOpType.max)
                        pst = ppst.tile([128, 128], F32, tag="pst")
                        nc.tensor.transpose(pst[:], h2c[:], wt["ident"][:])
                        hsl = h2t_sb[:, b * 128:(b + 1) * 128]
                        nc.scalar.copy(hsl, pst[:])
                        # fused [kv | q | skip] projection of the fresh
                        # chunk; biases preloaded into PSUM via Act
                        pall = ppall.tile([128, 512], F32, tag="pall")
                        nc.scalar.copy(pall[:], wt["ball"][:])
                        nc.tensor.matmul(pall[:], hsl, wt["Wall"][:],
                                         start=False, stop=True,
                                         skip_group_check=True)
                        nc.scalar.copy(kvst[:, b, :], pall[:, 0:256])
                        nc.scalar.copy(qst[:, b, :], pall[:, 256:384])
                        nc.scalar.copy(skip_sb[:, b, :], pall[:, 384:512])
                    nc.sync.dma_start(
                        kv_bounce[b0 * 128:(b0 + nbg) * 128, :]
                        .rearrange("(b p) m -> p b m", p=128),
                        kvst[:, b0:b0 + nbg, :])
                    if TIMING_1CORE:
                        nc.sync.dma_start(
                            kv_table[b0 * 128:(b0 + nbg) * 128, :],
                            kv_bounce[b0 * 128:(b0 + nbg) * 128, :])

            p1cm.__exit__(None, None, None)
            h2tm.__exit__(None, None, None)
            if not TIMING_1CORE:
                nc.gpsimd.collective_compute(
                    "AllGather",
                    mybir.AluOpType.bypass,
                    ins=[kv_bounce.ap().opt()],
                    outs=[kv_table.ap().opt()],
                    replica_groups=[list(range(W))],
                )

            # ---------------- Phase 2b: transformer aggregation ------------
            NG2 = -(-NB // G2B)
            with (
                tc.tile_pool(name="p2b_kv", bufs=4) as pkv,
                tc.tile_pool(name="p2b_oh", bufs=2) as poh2,
                tc.tile_pool(name="p2b_qe", bufs=2) as pqe,
                tc.tile_pool(name="p2b_qeps", bufs=1, space="PSUM") as pqeps,
                tc.tile_pool(name="p2b_pay", bufs=3) as ppay,
                tc.tile_pool(name="p2b_ex", bufs=6) as pex,
                tc.tile_pool(name="p2b_ps", bufs=2, space="PSUM") as pps3,
                tc.tile_pool(name="p2b_out", bufs=1) as pob,
            ):
              omst = pob.tile([128, NB, D], BF16, tag="omst")
              olst = pob.tile([128, NB, D], BF16, tag="olst")
              for _rep in range(REP.get('2b', REPEAT)):
                for g in range(NG2):
                    b0 = g * G2B
                    nbg = min(G2B, NB - b0)
                    kl = pkv.tile([128, nbg * T_LO, 256], BF16, tag="kl")
                    nc.gpsimd.dma_gather(
                        kl[:], kv_table[:],
                        il[:, b0 * T_LO * 8:(b0 + nbg) * T_LO * 8],
                        nbg * T_LO * 128, nbg * T_LO * 128, 256,
                        single_packet=False)
                    kh = pkv.tile([128, nbg * T_HI, 256], BF16, tag="kh")
                    nc.gpsimd.dma_gather(
                        kh[:], kv_table[HI0:, :],
                        ih[:, b0 * T_HI * 8:(b0 + nbg) * T_HI * 8],
                        nbg * T_HI * 128, nbg * T_HI * 128, 256,
                        single_packet=False)
                    oh = poh2.tile([128, nbg, T, 128], FP8, tag="oh2")
                    nc.sync.dma_start(
                        oh[:],
                        ohm_d[b0:b0 + nbg].rearrange("g p f -> p g f")
                        .rearrange("p g (t s) -> p g t s", s=128))
                    oht = poh2.tile([128, nbg, T, 128], FP8, tag="oht")
                    nc.sync.dma_start(
                        oht[:],
                        oht_d[b0:b0 + nbg].rearrange("g p f -> p g f")
                        .rearrange("p g (t s) -> p g t s", s=128))

                    for j in range(nbg):
                        b = b0 + j
                        qeps = pqeps.tile([128, T, 128], F32, tag="qeps")
                        for t in range(T):
                            nc.tensor.matmul(qeps[:, t, :], oht[:, j, t, :],
                                             qst[:, b, :],
                                             start=True, stop=True)
                        qe = pqe.tile([128, T, 128], BF16, tag="qe")
                        nc.scalar.copy(qe[:], qeps[:])

                        al = pex.tile([128, T, 2], F32, tag="al")
                        tm = ppay.tile([128, T, 128], FP16, tag="tm")
                        nc.vector.tensor_tensor(
                            tm[:, 0:T_LO, :], qe[:, 0:T_LO, :],
                            kl[:, j * T_LO:(j + 1) * T_LO, 0:128],
                            op=mybir.AluOpType.mult)
                        nc.vector.tensor_tensor(
                            tm[:, T_LO:T, :], qe[:, T_LO:T, :],
                            kh[:, j * T_HI:(j + 1) * T_HI, 0:128],
                            op=mybir.AluOpType.mult)
                        tmv = tm[:].rearrange("p t (c f) -> p t c f", c=2)
                        t2 = pex.tile([128, T, 2, 32], FP16, tag="t2")
                        nc.vector.tensor_tensor(
                            t2[:], tmv[:, :, :, 0:32], tmv[:, :, :, 32:64],
                            op=mybir.AluOpType.add)
                        t3 = pex.tile([128, T, 2, 16], FP16, tag="t3")
                        nc.vector.tensor_tensor(
                            t3[:], t2[:, :, :, 0:16], t2[:, :, :, 16:32],
                            op=mybir.AluOpType.add)
                        t4 = pex.tile([128, T, 2, 8], FP16, tag="t4")
                        nc.vector.tensor_tensor(
                            t4[:], t3[:, :, :, 0:8], t3[:, :, :, 8:16],
                            op=mybir.AluOpType.add)
                        nc.vector.tensor_reduce(
                            al[:], t4[:],
                            axis=mybir.AxisListType.X, op=mybir.AluOpType.add)
                        ex = pex.tile([128, T, 2], BF16, tag="ex")
                        nc.scalar.activation(ex[:], al[:],
                                             mybir.ActivationFunctionType.Exp,
                                             scale=0.125)

                        pay = ppay.tile([128, T, 130], BF16, tag="pay")
                        nc.vector.tensor_tensor(
                            pay[:, 0:T_LO, 0:128]
                            .rearrange("p t (f c) -> p t f c", c=2),
                            kl[:, j * T_LO:(j + 1) * T_LO, 128:256]
                            .rearrange("p t (f c) -> p t f c", c=2),
                            ex[:, 0:T_LO, :]
                            .rearrange("p t (one c) -> p t one c", one=1)
                            .broadcast_to([128, T_LO, 64, 2]),
                            op=mybir.AluOpType.mult)
                        nc.vector.tensor_tensor(
                            pay[:, T_LO:T, 0:128]
                            .rearrange("p t (f c) -> p t f c", c=2),
                            kh[:, j * T_HI:(j + 1) * T_HI, 128:256]
                            .rearrange("p t (f c) -> p t f c", c=2),
                            ex[:, T_LO:T, :]
                            .rearrange("p t (one c) -> p t one c", one=1)
                            .broadcast_to([128, T_HI, 64, 2]),
                            op=mybir.AluOpType.mult)
                        nc.scalar.copy(pay[:, :, 128:130], ex[:])

                        ps = pps3.tile([128, 130], F32, tag="ps2b")
                        for t in range(T):
                            nc.tensor.matmul(ps[:], oh[:, j, t, :],
                                             pay[:, t, :],
                                             start=(t == 0), stop=(t == T - 1))

                        den = pex.tile([128, 2], F32, tag="den")
                        nc.vector.tensor_scalar_add(den[:], ps[:, 128:130],
                                                    1e-16)
                        nc.vector.reciprocal(den[:], den[:])
                        psv = ps[:, 0:128].rearrange("p (f c) -> p f c", c=2)
                        nc.vector.scalar_tensor_tensor(
                            omst[:, b, :].rearrange("p (f one) -> p f one",
                                                    one=1),
                            psv[:, :, 0:1], den[:, 0:1],
                            skip_sb[:, b, 0:64]
                            .rearrange("p (f one) -> p f one", one=1),
                            op0=mybir.AluOpType.mult, op1=mybir.AluOpType.add)
                        nc.vector.scalar_tensor_tensor(
                            olst[:, b, :].rearrange("p (f one) -> p f one",
                                                    one=1),
                            psv[:, :, 1:2], den[:, 1:2],
                            skip_sb[:, b, 64:128]
                            .rearrange("p (f one) -> p f one", one=1),
                            op0=mybir.AluOpType.mult, op1=mybir.AluOpType.add)
                        nc.vector.tensor_scalar_min(olst[:, b, :],
                                                    olst[:, b, :], MAX_LOGSTD)
                    nc.sync.dma_start(
                        out_mu[b0 * 128:(b0 + nbg) * 128, :]
                        .rearrange("(b p) m -> p b m", p=128),
                        omst[:, b0:b0 + nbg, :])
                    nc.scalar.dma_start(
                        out_ls[b0 * 128:(b0 + nbg) * 128, :]
                        .rearrange("(b p) m -> p b m", p=128),
                        olst[:, b0:b0 + nbg, :])

    nc.compile()
    return nc


def make_weight_inputs(W_gcn, b_gcn, Wq_mu, bq_mu, Wk_mu, bk_mu, Wv_mu, bv_mu,
                       Ws_mu, bs_mu, Wq_ls, bq_ls, Wk_ls, bk_ls, Wv_ls, bv_ls,
                       Ws_ls, bs_ls):
    import ml_dtypes
    f = np.float32
    bf = ml_dtypes.bfloat16
    rep = lambda v: np.tile(np.asarray(v, f)[None, :], (128, 1))
    return dict(
        Wg0=np.ascontiguousarray(W_gcn[:128]).astype(bf),
        Wg1=np.ascontiguousarray(W_gcn[128:]).astype(bf),
        bgcn=rep(b_gcn),
        Wall=np.asarray(np.hstack(
            [Wk_mu, Wk_ls,
             np.stack([Wv_mu, Wv_ls], axis=2).reshape(Wv_mu.shape[0], -1),
             Wq_mu, Wq_ls, Ws_mu, Ws_ls]), bf),
        ball=rep(np.hstack([np.zeros(256, np.float32), bq_mu, bq_ls,
                            bs_mu + bv_mu, bs_ls + bv_ls])),
        ident=np.eye(128, dtype=f),
        iotab=np.tile(np.arange(128, dtype=f), (128, 1)).astype(bf),
    )


_CACHE = {}
PROFILE = False
LAST_EXEC_NS = None
REPEAT = 1      # re-run compute phases (device-time slope measurement)
REP = {}        # per-phase repeat override: {'1a':k,'1b':k,'2a':k,'2b':k}
TIMING_1CORE = False  # build single-core variant (collectives -> local copies)


def kernel(x, edge_index, **weights):
    in_maps, gperm, T_LO, T_HI = preprocess(x, edge_index)
    wmap = make_weight_inputs(**weights)
    for m in in_maps:
        m.update(wmap)

    key = (T_LO, T_HI)
    if key not in _CACHE:
        _CACHE[key] = build_kernel(T_LO, T_HI, weights)
    nc = _CACHE[key]

    global LAST_EXEC_NS
    res = run_bass_kernel_spmd(nc, in_maps, core_ids=list(range(W)),
                               trace=PROFILE)
    LAST_EXEC_NS = res.exec_time_ns

    mu = np.empty((N, D), np.float32)
    ls = np.empty((N, D), np.float32)
    for c in range(W):
        om = np.asarray(res.results[c]["out_mu"], np.float32)
        ol = np.asarray(res.results[c]["out_ls"], np.float32)
        gsel = gperm[c * NPAD:(c + 1) * NPAD]
        v = gsel >= 0
        mu[gsel[v]] = om[v]
        ls[gsel[v]] = ol[v]

    # bv is folded into the skip bias on-device (valid because softmax
    # weights sum to 1); nodes with zero in-degree have no attention term,
    # so undo the fold for them.
    dst = np.asarray(edge_index[1])
    indeg = np.bincount(dst, minlength=N)
    zd = np.nonzero(indeg == 0)[0]
    if zd.size:
        mu[zd] -= np.asarray(weights["bv_mu"], np.float32)
        ls[zd] = np.minimum(ls[zd] - np.asarray(weights["bv_ls"], np.float32),
                            MAX_LOGSTD)
    return mu, ls


# revision 73
# speedup vs baseline: 1.0070x; 1.0070x over previous
"""Trainium2 Bass kernel for nn_DimVariationalEmcoder (GCN + 2x TransformerConv VAE encoder).

Strategy (8 NeuronCores, SPMD):
  - Nodes sharded contiguously: core c owns global nodes [c*6250, (c+1)*6250),
    padded to 6272 = 49*128 local rows. Edges partitioned by destination core.
  - Per core, destination nodes are bin-packed into 49 blocks of 128 nodes with
    balanced in-edge counts; edges are grouped by block, sorted into "lo"
    (src table row < 32768) and "hi" tiles so dma_gather's int16 indices reach
    the whole 50176-row table via a rebased view.
  - Phase 1a: h' = (x @ W_gcn) * dinv for own nodes (bf16); AllGather the h'
    table so edge gathers are local.
  - Phase 1b: per 7-block group, batched dma_gather of h'[src] rows; per block,
    aggregate with a streamed fp8 one-hot (slot) matmul into PSUM, add
    self-loop term, * dinv + bias, LeakyReLU -> h2 chunk; transpose (PE) into
    a feature-major tile h2t_sb.
  - Phase 2a: own-stripe kv rows ([k_mu|k_ls|v interleaved], bf16), local q
    and skip tables from h2t_sb; AllGather the kv stripe (the halo exchange).
  - Phase 2b: per 2-block group, batched dma_gather of kv[src]; per block,
    expand q[dst] per edge on the PE (streamed transposed fp8 one-hot x local
    q block), alpha = q.k/8 via DVE mult+reduce, unshifted softmax (exp on
    Act), payload [ex*v | ex], aggregate via the one-hot matmul, normalize,
    add skip, clamp logstd. Pad edges have all-zero one-hot rows/cols, so no
    masking is needed anywhere.
"""

import numpy as np

import concourse.bacc as bacc
import concourse.mybir as mybir
import concourse.tile as tile
from concourse.bass_utils import run_bass_kernel_spmd

# Problem constants (hardcoded per the task contract).
N = 50000
E = 800000
F_IN = 256
H = 128
D = 64
W = 8                    # cores
NLOC = N // W            # 6250
NB = 49                  # blocks per core
NPAD = NB * 128          # 6272 padded local rows
G = W * NPAD             # 50176 global padded table rows
LO_LIM = 32768           # int16 gather index limit
MAX_LOGSTD = 10.0
LEAKY = 0.01
F32 = mybir.dt.float32
BF16 = mybir.dt.bfloat16
FP8 = mybir.dt.float8e4
FP16 = mybir.dt.float16
I16 = mybir.dt.int16
G1B = 7                  # blocks per gather group, phase 1b
G2B = 2                  # blocks per gather group, phase 2b


def configure(n, nb, lo_lim=32768):
    """Reconfigure problem size (for small-scale simulation tests)."""
    global N, NLOC, NB, NPAD, G, LO_LIM
    N = n
    NLOC = n // W
    NB = nb
    NPAD = nb * 128
    G = W * NPAD
    LO_LIM = lo_lim
    assert NLOC <= NPAD


# ----------------------------------------------------------------------------
# Host-side preprocessing
# ----------------------------------------------------------------------------

def _pack_blocks(indeg):
    """Greedy balanced bin-packing of NPAD nodes into NB bins of 128 nodes."""
    import heapq
    order = np.argsort(-indeg, kind="stable")
    bin_cnt = np.zeros(NB, np.int64)
    heap = [(0, b) for b in range(NB)]
    heapq.heapify(heap)
    assign = np.empty(NPAD, np.int64)
    for v in order:
        while True:
            e, b = heapq.heappop(heap)
            if bin_cnt[b] < 128:
                break
        assign[v] = b
        bin_cnt[b] += 1
        heapq.heappush(heap, (e + int(indeg[v]), b))
    perm = np.empty(NPAD, np.int64)
    slot_of = np.empty(NPAD, np.int64)
    fill = np.zeros(NB, np.int64)
    for v in range(NPAD):
        b = assign[v]
        perm[b * 128 + fill[b]] = v
        slot_of[v] = fill[b]
        fill[b] += 1
    return perm, assign, slot_of


def _wrap_idx(a):
    """[L] int array -> [128, L//16] int16 wrapped layout (replicated x8)."""
    w = np.ascontiguousarray(a.reshape(-1, 16).T.astype(np.int16))
    return np.tile(w, (8, 1))


def preprocess(x, edge_index):
    import ml_dtypes
    fp8 = ml_dtypes.float8_e4m3

    src = np.asarray(edge_index[0], dtype=np.int64)
    dst = np.asarray(edge_index[1], dtype=np.int64)
    x = np.asarray(x, dtype=np.float32)

    deg = np.bincount(dst, minlength=N).astype(np.float64) + 1.0
    dinv = (1.0 / np.sqrt(deg)).astype(np.float32)

    # Per-core permutations (destination-side bin packing).
    perms = []
    for c in range(W):
        mask = (dst // NLOC) == c
        dl = dst[mask] - c * NLOC
        indeg = np.bincount(dl, minlength=NPAD).astype(np.int64)
        p, a, s = _pack_blocks(indeg)
        perms.append(p)

    # Global padded table position of every real node.
    pos_of = np.empty(N, np.int64)
    for c in range(W):
        inv = np.empty(NPAD, np.int64)
        inv[perms[c]] = np.arange(NPAD)
        pos_of[c * NLOC:(c + 1) * NLOC] = c * NPAD + inv[:NLOC]

    src_pos = pos_of[src]

    # First pass: per-(core, block) lo/hi counts to size the tile grid.
    per_core = []
    t_lo_max, t_hi_max = 0, 0
    for c in range(W):
        mask = (dst // NLOC) == c
        e_src_pos = src_pos[mask]
        e_dl = dst[mask] - c * NLOC
        e_nl = np.empty(NPAD, np.int64)
        e_nl[perms[c]] = np.arange(NPAD)
        new_local = e_nl[e_dl]
        blk = new_local // 128
        slo = new_local % 128
        is_hi = e_src_pos >= LO_LIM
        per_core.append((e_src_pos, blk, slo, is_hi))
        for b in range(NB):
            m = blk == b
            nlo = int(np.sum(m & ~is_hi))
            nhi = int(np.sum(m & is_hi))
            t_lo_max = max(t_lo_max, -(-nlo // 128))
            t_hi_max = max(t_hi_max, -(-nhi // 128))

    T_LO, T_HI = max(t_lo_max, 1), max(t_hi_max, 1)
    T = T_LO + T_HI

    in_maps = []
    gperm_cores = []
    for c in range(W):
        e_src_pos, blk, slo, is_hi = per_core[c]
        idx_lo = np.zeros((NB, T_LO * 128), np.int64)
        idx_hi = np.zeros((NB, T_HI * 128), np.int64)
        slot_a = np.full((NB, T * 128), 128, np.int64)  # pad slot -> no onehot
        for b in range(NB):
            m = blk == b
            for hi in (False, True):
                mm = m & (is_hi if hi else ~is_hi)
                k = int(mm.sum())
                if hi:
                    idx_hi[b, :k] = e_src_pos[mm] - LO_LIM
                    off = T_LO * 128
                else:
                    idx_lo[b, :k] = e_src_pos[mm]
                    off = 0
                slot_a[b, off:off + k] = slo[mm]

        # one-hot [NB, 128(e), T*128(slot-major)] and its transpose, fp8
        sa = slot_a.reshape(NB, T, 128)
        rng = np.arange(128)
        oh_np = (sa.transpose(0, 2, 1)[:, :, :, None] == rng[None, None, None, :])
        oh_np = np.ascontiguousarray(
            oh_np.reshape(NB, 128, T * 128)).astype(fp8)
        oht_np = (sa[:, None, :, :] == rng[None, :, None, None])
        oht_np = np.ascontiguousarray(
            oht_np.reshape(NB, 128, T * 128)).astype(fp8)

        # per-core x slice in permuted order, chunk-transposed, bf16
        gsel = np.where(perms[c] < NLOC, c * NLOC + perms[c], -1)
        gperm_cores.append(gsel)
        x_own = np.zeros((NPAD, F_IN), np.float32)
        vv = gsel >= 0
        x_own[vv] = x[gsel[vv]]
        x_ownT = np.ascontiguousarray(
            x_own.reshape(NB, 128, F_IN).transpose(0, 2, 1)).astype(
                ml_dtypes.bfloat16)

        dv = np.zeros(NPAD, np.float32)
        sel_src = np.where(perms[c] < NLOC, c * NLOC + perms[c], 0)
        dv[vv] = dinv[sel_src[vv]]
        dinvn_sb = np.ascontiguousarray(dv.reshape(NB, 128).T)

        lane = np.ascontiguousarray(
            slot_a.reshape(NB * T, 128).T).astype(ml_dtypes.bfloat16)
        in_maps.append(dict(
            x_ownT=x_ownT,
            dinvn=dinvn_sb,
            idx_lo=_wrap_idx(idx_lo.reshape(-1)),
            idx_hi=_wrap_idx(idx_hi.reshape(-1)),
            eslot=lane,
            ohm=oh_np,
            oht=oht_np,
        ))

    gperm = np.concatenate(gperm_cores)          # [G] global node id or -1
    return in_maps, gperm, T_LO, T_HI


# ----------------------------------------------------------------------------
# Kernel build
# ----------------------------------------------------------------------------

def build_kernel(T_LO, T_HI, weights):
    T = T_LO + T_HI
    HI0 = LO_LIM if G > LO_LIM else 0  # hi-gather rebase offset
    ndev = 1 if TIMING_1CORE else W
    nc = bacc.Bacc("TRN2", target_bir_lowering=False, debug=False,
                   num_devices=ndev)

    # inputs
    x_ownT = nc.dram_tensor("x_ownT", [NB, F_IN, 128], BF16, kind="ExternalInput")
    dinvn_d = nc.dram_tensor("dinvn", [128, NB], F32, kind="ExternalInput")
    idx_lo_d = nc.dram_tensor("idx_lo", [128, NB * T_LO * 8], I16, kind="ExternalInput")
    idx_hi_d = nc.dram_tensor("idx_hi", [128, NB * T_HI * 8], I16, kind="ExternalInput")
    ohm_d = nc.dram_tensor("ohm", [NB, 128, T * 128], FP8, kind="ExternalInput")
    oht_d = nc.dram_tensor("oht", [NB, 128, T * 128], FP8, kind="ExternalInput")
    eslot_d = nc.dram_tensor("eslot", [128, NB * T], BF16, kind="ExternalInput")
    wnames = ["Wall", "ball", "ident", "iotab"]
    bf16_w = {"Wall", "iotab"}
    w1names = ["Wg0", "Wg1", "bgcn"]
    bf16_w1 = {"Wg0", "Wg1"}
    wshapes = {
        "Wg0": [128, H], "Wg1": [128, H], "bgcn": [128, H],
        "Wall": [H, 512], "ball": [128, 512],
        "ident": [128, 128], "iotab": [128, 128],
    }
    wd = {k: nc.dram_tensor(k, wshapes[k],
                            BF16 if (k in bf16_w or k in bf16_w1) else F32,
                            kind="ExternalInput")
          for k in wnames + w1names}

    out_mu = nc.dram_tensor("out_mu", [NPAD, D], BF16, kind="ExternalOutput")
    out_ls = nc.dram_tensor("out_ls", [NPAD, D], BF16, kind="ExternalOutput")

    # internal DRAM
    hw_bounce = nc.dram_tensor("hw_bounce", [NPAD, H], BF16)
    hw_table = nc.dram_tensor("hw_table", [G, H], BF16, addr_space="Shared")
    kv_bounce = nc.dram_tensor("kv_bounce", [NPAD, 256], BF16)
    kv_table = nc.dram_tensor("kv_table", [G, 256], BF16, addr_space="Shared")

    with tile.TileContext(nc) as tc:
        with (
            tc.tile_pool(name="const", bufs=1) as cp,
            tc.tile_pool(name="persist", bufs=1) as pp,
        ):
            # cross-phase constants
            wt = {}
            for k in wnames:
                t = cp.tile(wshapes[k], BF16 if k in bf16_w else F32, tag=k)
                nc.sync.dma_start(t[:], wd[k][:])
                wt[k] = t
            il = cp.tile([128, NB * T_LO * 8], I16, tag="il")
            nc.sync.dma_start(il[:], idx_lo_d[:])
            ih = cp.tile([128, NB * T_HI * 8], I16, tag="ih")
            nc.sync.dma_start(ih[:], idx_hi_d[:])
            eslot = cp.tile([128, NB * T], BF16, tag="eslot")
            nc.sync.dma_start(eslot[:], eslot_d[:])


            skip_sb = pp.tile([128, NB, 128], BF16, tag="skip")
            qst = pp.tile([128, NB, 128], BF16, tag="qst")

            # ---- Phase 1: GCN ------------------------------------------
            h2tm = tc.tile_pool(name="h2t_pool", bufs=1)
            hp = h2tm.__enter__()
            h2t_sb = hp.tile([128, NPAD], BF16, tag="h2t")
            kvst = hp.tile([128, NB, 256], BF16, tag="kvst")
            p1cm = tc.tile_pool(name="p1const", bufs=1)
            p1cp = p1cm.__enter__()
            for k in w1names:
                t = p1cp.tile(wshapes[k], BF16 if k in bf16_w1 else F32, tag=k)
                nc.sync.dma_start(t[:], wd[k][:])
                wt[k] = t
            dinvn = p1cp.tile([128, NB], F32, tag="dinvn")
            nc.sync.dma_start(dinvn[:], dinvn_d[:])
            hwo = p1cp.tile([128, NB, H], BF16, tag="hwo")

            # ------- Phase 1a: own h' slice, then AllGather the table ----
            with (
                tc.tile_pool(name="p1a_in", bufs=5) as pin,
                tc.tile_pool(name="p1a_ps", bufs=2, space="PSUM") as pps,
            ):
              for _rep in range(REP.get('1a', REPEAT)):
                for g in range(-(-NB // G1B)):
                    b0 = g * G1B
                    nbg = min(G1B, NB - b0)
                    xt = pin.tile([128, nbg, 2, 128], BF16, tag="xt")
                    nc.scalar.dma_start(
                        xt[:], x_ownT[b0:b0 + nbg]
                        .rearrange("g (j p) m -> p g j m", p=128))
                    for j in range(nbg):
                        b = b0 + j
                        ps = pps.tile([128, H], F32, tag="ps")
                        nc.tensor.matmul(ps[:], xt[:, j, 0, :], wt["Wg0"][:],
                                         start=True, stop=False)
                        nc.tensor.matmul(ps[:], xt[:, j, 1, :], wt["Wg1"][:],
                                         start=False, stop=True)
                        # h' = (x @ Wg) * dinv  (norm factored per-node)
                        nc.scalar.mul(hwo[:, b, :], ps[:], dinvn[:, b:b + 1])
                    nc.sync.dma_start(
                        hw_bounce[b0 * 128:(b0 + nbg) * 128, :]
                        .rearrange("(b p) m -> p b m", p=128),
                        hwo[:, b0:b0 + nbg, :])
                    if TIMING_1CORE:
                        nc.sync.dma_start(
                            hw_table[b0 * 128:(b0 + nbg) * 128, :],
                            hw_bounce[b0 * 128:(b0 + nbg) * 128, :])
            if not TIMING_1CORE:
                nc.gpsimd.collective_compute(
                    "AllGather",
                    mybir.AluOpType.bypass,
                    ins=[hw_bounce.ap().opt()],
                    outs=[hw_table.ap().opt()],
                    replica_groups=[list(range(W))],
                )

            # ------- Phase 1b: GCN aggregation + fused kv/q/skip -----------
            NG1 = -(-NB // G1B)
            with (
                tc.tile_pool(name="p1b_g", bufs=3) as pg,
                tc.tile_pool(name="p1b_oh", bufs=4) as poh,
                tc.tile_pool(name="p1b_ps", bufs=3, space="PSUM") as pps1,
                tc.tile_pool(name="p1b_pst", bufs=2, space="PSUM") as ppst,
                tc.tile_pool(name="p1b_pall", bufs=2, space="PSUM") as ppall,
                tc.tile_pool(name="p1b_h2", bufs=4) as ph2,
            ):
              for _rep in range(REP.get('1b', REPEAT)):
                for g in range(NG1):
                    b0 = g * G1B
                    nbg = min(G1B, NB - b0)
                    gl = pg.tile([128, nbg * T_LO, H], BF16, tag="gl")
                    SPL = 3
                    for c0 in range(0, nbg, SPL):
                        cn = min(SPL, nbg - c0)
                        nc.gpsimd.dma_gather(
                            gl[:, c0 * T_LO:(c0 + cn) * T_LO, :],
                            hw_table[:],
                            il[:, (b0 + c0) * T_LO * 8:
                               (b0 + c0 + cn) * T_LO * 8],
                            cn * T_LO * 128, cn * T_LO * 128, H,
                            single_packet=False)
                    gh = pg.tile([128, nbg * T_HI, H], BF16, tag="gh")
                    for c0 in range(0, nbg, SPL):
                        cn = min(SPL, nbg - c0)
                        nc.gpsimd.dma_gather(
                            gh[:, c0 * T_HI:(c0 + cn) * T_HI, :],
                            hw_table[HI0:, :],
                            ih[:, (b0 + c0) * T_HI * 8:
                               (b0 + c0 + cn) * T_HI * 8],
                            cn * T_HI * 128, cn * T_HI * 128, H,
                            single_packet=False)
                    for j in range(nbg):
                        b = b0 + j
                        oh1 = poh.tile([128, T, 128], BF16, tag="oh1")
                        esl = eslot[:, b * T:(b + 1) * T]
                        nc.vector.tensor_tensor(
                            oh1[:],
                            esl.rearrange("p (c one) -> p c one", one=1)
                            .broadcast_to([128, T, 128]),
                            wt["iotab"][:]
                            .rearrange("p (one s) -> p one s", one=1)
                            .broadcast_to([128, T, 128]),
                            op=mybir.AluOpType.is_equal)
                        ps1 = pps1.tile([128, H], F32, tag="ps1")
                        for t in range(T_LO):
                            nc.tensor.matmul(ps1[:], oh1[:, t, :],
                                             gl[:, j * T_LO + t, :],
                                             start=(t == 0), stop=False)
                        for t in range(T_HI):
                            nc.tensor.matmul(ps1[:], oh1[:, T_LO + t, :],
                                             gh[:, j * T_HI + t, :],
                                             start=False, stop=(t == T_HI - 1))
                        # h2 = leaky((agg + h'_own) * dinv + bias)
                        h2c = ph2.tile([128, H], F32, tag="h2c")
                        nc.vector.tensor_tensor(h2c[:], ps1[:], hwo[:, b, :],
                                                op=mybir.AluOpType.add)
                        nc.vector.scalar_tensor_tensor(
                            h2c[:], h2c[:], dinvn[:, b:b + 1], wt["bgcn"][:],
                            op0=mybir.AluOpType.mult, op1=mybir.AluOpType.add)
                        nc.vector.scalar_tensor_tensor(
                            h2c[:], h2c[:], LEAKY, h2c[:],
                            op0=mybir.AluOpType.mult, op1=mybir.AluOpType.max)
                        pst = ppst.tile([128, 128], F32, tag="pst")
                        nc.tensor.transpose(pst[:], h2c[:], wt["ident"][:])
                        hsl = h2t_sb[:, b * 128:(b + 1) * 128]
                        nc.scalar.copy(hsl, pst[:])
                        # fused [kv | q | skip] projection of the fresh
                        # chunk; biases preloaded into PSUM via Act
                        pall = ppall.tile([128, 512], F32, tag="pall")
                        nc.scalar.copy(pall[:], wt["ball"][:])
                        nc.tensor.matmul(pall[:], hsl, wt["Wall"][:],
                                         start=False, stop=True,
                                         skip_group_check=True)
                        nc.scalar.copy(kvst[:, b, :], pall[:, 0:256])
                        nc.scalar.copy(qst[:, b, :], pall[:, 256:384])
                        nc.scalar.copy(skip_sb[:, b, :], pall[:, 384:512])
                    nc.sync.dma_start(
                        kv_bounce[b0 * 128:(b0 + nbg) * 128, :]
                        .rearrange("(b p) m -> p b m", p=128),
                        kvst[:, b0:b0 + nbg, :])
                    if TIMING_1CORE:
                        nc.sync.dma_start(
                            kv_table[b0 * 128:(b0 + nbg) * 128, :],
                            kv_bounce[b0 * 128:(b0 + nbg) * 128, :])

            p1cm.__exit__(None, None, None)
            h2tm.__exit__(None, None, None)
            if not TIMING_1CORE:
                nc.gpsimd.collective_compute(
                    "AllGather",
                    mybir.AluOpType.bypass,
                    ins=[kv_bounce.ap().opt()],
                    outs=[kv_table.ap().opt()],
                    replica_groups=[list(range(W))],
                )

            # ---------------- Phase 2b: transformer aggregation ------------
            NG2 = -(-NB // G2B)
            with (
                tc.tile_pool(name="p2b_kv", bufs=4) as pkv,
                tc.tile_pool(name="p2b_oh", bufs=2) as poh2,
                tc.tile_pool(name="p2b_qe", bufs=2) as pqe,
                tc.tile_pool(name="p2b_qeps", bufs=1, space="PSUM") as pqeps,
                tc.tile_pool(name="p2b_pay", bufs=3) as ppay,
                tc.tile_pool(name="p2b_ex", bufs=6) as pex,
                tc.tile_pool(name="p2b_ps", bufs=2, space="PSUM") as pps3,
                tc.tile_pool(name="p2b_out", bufs=1) as pob,
            ):
              omst = pob.tile([128, NB, D], BF16, tag="omst")
              olst = pob.tile([128, NB, D], BF16, tag="olst")
              for _rep in range(REP.get('2b', REPEAT)):
                for g in range(NG2):
                    b0 = g * G2B
                    nbg = min(G2B, NB - b0)
                    kl = pkv.tile([128, nbg * T_LO, 256], BF16, tag="kl")
                    nc.gpsimd.dma_gather(
                        kl[:], kv_table[:],
                        il[:, b0 * T_LO * 8:(b0 + nbg) * T_LO * 8],
                        nbg * T_LO * 128, nbg * T_LO * 128, 256,
                        single_packet=False)
                    kh = pkv.tile([128, nbg * T_HI, 256], BF16, tag="kh")
                    nc.gpsimd.dma_gather(
                        kh[:], kv_table[HI0:, :],
                        ih[:, b0 * T_HI * 8:(b0 + nbg) * T_HI * 8],
                        nbg * T_HI * 128, nbg * T_HI * 128, 256,
                        single_packet=False)
                    oh = poh2.tile([128, nbg, T, 128], FP8, tag="oh2")
                    nc.sync.dma_start(
                        oh[:],
                        ohm_d[b0:b0 + nbg].rearrange("g p f -> p g f")
                        .rearrange("p g (t s) -> p g t s", s=128))
                    oht = poh2.tile([128, nbg, T, 128], FP8, tag="oht")
                    nc.sync.dma_start(
                        oht[:],
                        oht_d[b0:b0 + nbg].rearrange("g p f -> p g f")
                        .rearrange("p g (t s) -> p g t s", s=128))

                    for j in range(nbg):
                        b = b0 + j
                        qeps = pqeps.tile([128, T, 128], F32, tag="qeps")
                        for t in range(T):
                            nc.tensor.matmul(qeps[:, t, :], oht[:, j, t, :],
                                             qst[:, b, :],
                                             start=True, stop=True)
                        qe = pqe.tile([128, T, 128], BF16, tag="qe")
                        nc.scalar.copy(qe[:], qeps[:])

                        al = pex.tile([128, T, 2], F32, tag="al")
                        tm = ppay.tile([128, T, 128], FP16, tag="tm")
                        nc.vector.tensor_tensor(
                            tm[:, 0:T_LO, :], qe[:, 0:T_LO, :],
                            kl[:, j * T_LO:(j + 1) * T_LO, 0:128],
                            op=mybir.AluOpType.mult)
                        nc.vector.tensor_tensor(
                            tm[:, T_LO:T, :], qe[:, T_LO:T, :],
                            kh[:, j * T_HI:(j + 1) * T_HI, 0:128],
                            op=mybir.AluOpType.mult)
                        tmv = tm[:].rearrange("p t (c f) -> p t c f", c=2)
                        t2 = pex.tile([128, T, 2, 32], FP16, tag="t2")
                        nc.vector.tensor_tensor(
                            t2[:], tmv[:, :, :, 0:32], tmv[:, :, :, 32:64],
                            op=mybir.AluOpType.add)
                        t3 = pex.tile([128, T, 2, 16], FP16, tag="t3")
                        nc.vector.tensor_tensor(
                            t3[:], t2[:, :, :, 0:16], t2[:, :, :, 16:32],
                            op=mybir.AluOpType.add)
                        t4 = pex.tile([128, T, 2, 8], FP16, tag="t4")
                        nc.vector.tensor_tensor(
                            t4[:], t3[:, :, :, 0:8], t3[:, :, :, 8:16],
                            op=mybir.AluOpType.add)
                        nc.vector.tensor_reduce(
                            al[:], t4[:],
                            axis=mybir.AxisListType.X, op=mybir.AluOpType.add)
                        ex = pex.tile([128, T, 2], BF16, tag="ex")
                        nc.scalar.activation(ex[:], al[:],
                                             mybir.ActivationFunctionType.Exp,
                                             scale=0.125)

                        pay = ppay.tile([128, T, 130], BF16, tag="pay")
                        nc.vector.tensor_tensor(
                            pay[:, 0:T_LO, 0:128]
                            .rearrange("p t (f c) -> p t f c", c=2),
                            kl[:, j * T_LO:(j + 1) * T_LO, 128:256]
                            .rearrange("p t (f c) -> p t f c", c=2),
                            ex[:, 0:T_LO, :]
                            .rearrange("p t (one c) -> p t one c", one=1)
                            .broadcast_to([128, T_LO, 64, 2]),
                            op=mybir.AluOpType.mult)
                        nc.vector.tensor_tensor(
                            pay[:, T_LO:T, 0:128]
                            .rearrange("p t (f c) -> p t f c", c=2),
                            kh[:, j * T_HI:(j + 1) * T_HI, 128:256]
                            .rearrange("p t (f c) -> p t f c", c=2),
                            ex[:, T_LO:T, :]
                            .rearrange("p t (one c) -> p t one c", one=1)
                            .broadcast_to([128, T_HI, 64, 2]),
                            op=mybir.AluOpType.mult)
                        nc.scalar.copy(pay[:, :, 128:130], ex[:])

                        ps = pps3.tile([128, 130], F32, tag="ps2b")
                        for t in range(T):
                            nc.tensor.matmul(ps[:], oh[:, j, t, :],
                                             pay[:, t, :],
                                             start=(t == 0), stop=(t == T - 1))

                        den = pex.tile([128, 2], F32, tag="den")
                        nc.vector.tensor_scalar_add(den[:], ps[:, 128:130],
                                                    1e-16)
                        nc.vector.reciprocal(den[:], den[:])
                        psv = ps[:, 0:128].rearrange("p (f c) -> p f c", c=2)
                        nc.vector.scalar_tensor_tensor(
                            omst[:, b, :].rearrange("p (f one) -> p f one",
                                                    one=1),
                            psv[:, :, 0:1], den[:, 0:1],
                            skip_sb[:, b, 0:64]
                            .rearrange("p (f one) -> p f one", one=1),
                            op0=mybir.AluOpType.mult, op1=mybir.AluOpType.add)
                        nc.vector.scalar_tensor_tensor(
                            olst[:, b, :].rearrange("p (f one) -> p f one",
                                                    one=1),
                            psv[:, :, 1:2], den[:, 1:2],
                            skip_sb[:, b, 64:128]
                            .rearrange("p (f one) -> p f one", one=1),
                            op0=mybir.AluOpType.mult, op1=mybir.AluOpType.add)
                        nc.vector.tensor_scalar_min(olst[:, b, :],
                                                    olst[:, b, :], MAX_LOGSTD)
                    nc.sync.dma_start(
                        out_mu[b0 * 128:(b0 + nbg) * 128, :]
                        .rearrange("(b p) m -> p b m", p=128),
                        omst[:, b0:b0 + nbg, :])
                    nc.scalar.dma_start(
                        out_ls[b0 * 128:(b0 + nbg) * 128, :]
                        .rearrange("(b p) m -> p b m", p=128),
                        olst[:, b0:b0 + nbg, :])

    nc.compile()
    return nc


def make_weight_inputs(W_gcn, b_gcn, Wq_mu, bq_mu, Wk_mu, bk_mu, Wv_mu, bv_mu,
                       Ws_mu, bs_mu, Wq_ls, bq_ls, Wk_ls, bk_ls, Wv_ls, bv_ls,
                       Ws_ls, bs_ls):
    import ml_dtypes
    f = np.float32
    bf = ml_dtypes.bfloat16
    rep = lambda v: np.tile(np.asarray(v, f)[None, :], (128, 1))
    return dict(
        Wg0=np.ascontiguousarray(W_gcn[:128]).astype(bf),
        Wg1=np.ascontiguousarray(W_gcn[128:]).astype(bf),
        bgcn=rep(b_gcn),
        Wall=np.asarray(np.hstack(
            [Wk_mu, Wk_ls,
             np.stack([Wv_mu, Wv_ls], axis=2).reshape(Wv_mu.shape[0], -1),
             Wq_mu, Wq_ls, Ws_mu, Ws_ls]), bf),
        ball=rep(np.hstack([np.zeros(256, np.float32), bq_mu, bq_ls,
                            bs_mu + bv_mu, bs_ls + bv_ls])),
        ident=np.eye(128, dtype=f),
        iotab=np.tile(np.arange(128, dtype=f), (128, 1)).astype(bf),
    )


_CACHE = {}
PROFILE = False
LAST_EXEC_NS = None
REPEAT = 1      # re-run compute phases (device-time slope measurement)
REP = {}        # per-phase repeat override: {'1a':k,'1b':k,'2a':k,'2b':k}
TIMING_1CORE = False  # build single-core variant (collectives -> local copies)


def kernel(x, edge_index, **weights):
    in_maps, gperm, T_LO, T_HI = preprocess(x, edge_index)
    wmap = make_weight_inputs(**weights)
    for m in in_maps:
        m.update(wmap)

    key = (T_LO, T_HI)
    if key not in _CACHE:
        _CACHE[key] = build_kernel(T_LO, T_HI, weights)
    nc = _CACHE[key]

    global LAST_EXEC_NS
    res = run_bass_kernel_spmd(nc, in_maps, core_ids=list(range(W)),
                               trace=PROFILE)
    LAST_EXEC_NS = res.exec_time_ns

    mu = np.empty((N, D), np.float32)
    ls = np.empty((N, D), np.float32)
    for c in range(W):
        om = np.asarray(res.results[c]["out_mu"], np.float32)
        ol = np.asarray(res.results[c]["out_ls"], np.float32)
        gsel = gperm[c * NPAD:(c + 1) * NPAD]
        v = gsel >= 0
        mu[gsel[v]] = om[v]
        ls[gsel[v]] = ol[v]

    # bv is folded into the skip bias on-device (valid because softmax
    # weights sum to 1); nodes with zero in-degree have no attention term,
    # so undo the fold for them.
    dst = np.asarray(edge_index[1])
    indeg = np.bincount(dst, minlength=N)
    zd = np.nonzero(indeg == 0)[0]
    if zd.size:
        mu[zd] -= np.asarray(weights["bv_mu"], np.float32)
        ls[zd] = np.minimum(ls[zd] - np.asarray(weights["bv_ls"], np.float32),
                            MAX_LOGSTD)
    return mu, ls


# revision 74
# speedup vs baseline: 1.0114x; 1.0044x over previous
"""Trainium2 Bass kernel for nn_DimVariationalEmcoder (GCN + 2x TransformerConv VAE encoder).

Strategy (8 NeuronCores, SPMD):
  - Nodes sharded contiguously: core c owns global nodes [c*6250, (c+1)*6250),
    padded to 6272 = 49*128 local rows. Edges partitioned by destination core.
  - Per core, destination nodes are bin-packed into 49 blocks of 128 nodes with
    balanced in-edge counts; edges are grouped by block, sorted into "lo"
    (src table row < 32768) and "hi" tiles so dma_gather's int16 indices reach
    the whole 50176-row table via a rebased view.
  - Phase 1a: h' = (x @ W_gcn) * dinv for own nodes (bf16); AllGather the h'
    table so edge gathers are local.
  - Phase 1b: per 7-block group, batched dma_gather of h'[src] rows; per block,
    aggregate with a streamed fp8 one-hot (slot) matmul into PSUM, add
    self-loop term, * dinv + bias, LeakyReLU -> h2 chunk; transpose (PE) into
    a feature-major tile h2t_sb.
  - Phase 2a: own-stripe kv rows ([k_mu|k_ls|v interleaved], bf16), local q
    and skip tables from h2t_sb; AllGather the kv stripe (the halo exchange).
  - Phase 2b: per 2-block group, batched dma_gather of kv[src]; per block,
    expand q[dst] per edge on the PE (streamed transposed fp8 one-hot x local
    q block), alpha = q.k/8 via DVE mult+reduce, unshifted softmax (exp on
    Act), payload [ex*v | ex], aggregate via the one-hot matmul, normalize,
    add skip, clamp logstd. Pad edges have all-zero one-hot rows/cols, so no
    masking is needed anywhere.
"""

import numpy as np

import concourse.bacc as bacc
import concourse.mybir as mybir
import concourse.tile as tile
from concourse.bass_utils import run_bass_kernel_spmd

# Problem constants (hardcoded per the task contract).
N = 50000
E = 800000
F_IN = 256
H = 128
D = 64
W = 8                    # cores
NLOC = N // W            # 6250
NB = 49                  # blocks per core
NPAD = NB * 128          # 6272 padded local rows
G = W * NPAD             # 50176 global padded table rows
LO_LIM = 32768           # int16 gather index limit
MAX_LOGSTD = 10.0
LEAKY = 0.01
F32 = mybir.dt.float32
BF16 = mybir.dt.bfloat16
FP8 = mybir.dt.float8e4
FP16 = mybir.dt.float16
I16 = mybir.dt.int16
G1B = 7                  # blocks per gather group, phase 1b
G2B = 2                  # blocks per gather group, phase 2b


def configure(n, nb, lo_lim=32768):
    """Reconfigure problem size (for small-scale simulation tests)."""
    global N, NLOC, NB, NPAD, G, LO_LIM
    N = n
    NLOC = n // W
    NB = nb
    NPAD = nb * 128
    G = W * NPAD
    LO_LIM = lo_lim
    assert NLOC <= NPAD


# ----------------------------------------------------------------------------
# Host-side preprocessing
# ----------------------------------------------------------------------------

def _pack_blocks(indeg):
    """Greedy balanced bin-packing of NPAD nodes into NB bins of 128 nodes."""
    import heapq
    order = np.argsort(-indeg, kind="stable")
    bin_cnt = np.zeros(NB, np.int64)
    heap = [(0, b) for b in range(NB)]
    heapq.heapify(heap)
    assign = np.empty(NPAD, np.int64)
    for v in order:
        while True:
            e, b = heapq.heappop(heap)
            if bin_cnt[b] < 128:
                break
        assign[v] = b
        bin_cnt[b] += 1
        heapq.heappush(heap, (e + int(indeg[v]), b))
    perm = np.empty(NPAD, np.int64)
    slot_of = np.empty(NPAD, np.int64)
    fill = np.zeros(NB, np.int64)
    for v in range(NPAD):
        b = assign[v]
        perm[b * 128 + fill[b]] = v
        slot_of[v] = fill[b]
        fill[b] += 1
    return perm, assign, slot_of


def _wrap_idx(a):
    """[L] int array -> [128, L//16] int16 wrapped layout (replicated x8)."""
    w = np.ascontiguousarray(a.reshape(-1, 16).T.astype(np.int16))
    return np.tile(w, (8, 1))


def preprocess(x, edge_index):
    import ml_dtypes
    fp8 = ml_dtypes.float8_e4m3

    src = np.asarray(edge_index[0], dtype=np.int64)
    dst = np.asarray(edge_index[1], dtype=np.int64)
    x = np.asarray(x, dtype=np.float32)

    deg = np.bincount(dst, minlength=N).astype(np.float64) + 1.0
    dinv = (1.0 / np.sqrt(deg)).astype(np.float32)

    # Per-core permutations (destination-side bin packing).
    perms = []
    for c in range(W):
        mask = (dst // NLOC) == c
        dl = dst[mask] - c * NLOC
        indeg = np.bincount(dl, minlength=NPAD).astype(np.int64)
        p, a, s = _pack_blocks(indeg)
        perms.append(p)

    # Global padded table position of every real node.
    pos_of = np.empty(N, np.int64)
    for c in range(W):
        inv = np.empty(NPAD, np.int64)
        inv[perms[c]] = np.arange(NPAD)
        pos_of[c * NLOC:(c + 1) * NLOC] = c * NPAD + inv[:NLOC]

    src_pos = pos_of[src]

    # First pass: per-(core, block) lo/hi counts to size the tile grid.
    per_core = []
    t_lo_max, t_hi_max = 0, 0
    for c in range(W):
        mask = (dst // NLOC) == c
        e_src_pos = src_pos[mask]
        e_dl = dst[mask] - c * NLOC
        e_nl = np.empty(NPAD, np.int64)
        e_nl[perms[c]] = np.arange(NPAD)
        new_local = e_nl[e_dl]
        blk = new_local // 128
        slo = new_local % 128
        is_hi = e_src_pos >= LO_LIM
        per_core.append((e_src_pos, blk, slo, is_hi))
        for b in range(NB):
            m = blk == b
            nlo = int(np.sum(m & ~is_hi))
            nhi = int(np.sum(m & is_hi))
            t_lo_max = max(t_lo_max, -(-nlo // 128))
            t_hi_max = max(t_hi_max, -(-nhi // 128))

    T_LO, T_HI = max(t_lo_max, 1), max(t_hi_max, 1)
    T = T_LO + T_HI

    in_maps = []
    gperm_cores = []
    for c in range(W):
        e_src_pos, blk, slo, is_hi = per_core[c]
        idx_lo = np.zeros((NB, T_LO * 128), np.int64)
        idx_hi = np.zeros((NB, T_HI * 128), np.int64)
        slot_a = np.full((NB, T * 128), 128, np.int64)  # pad slot -> no onehot
        for b in range(NB):
            m = blk == b
            for hi in (False, True):
                mm = m & (is_hi if hi else ~is_hi)
                k = int(mm.sum())
                if hi:
                    idx_hi[b, :k] = e_src_pos[mm] - LO_LIM
                    off = T_LO * 128
                else:
                    idx_lo[b, :k] = e_src_pos[mm]
                    off = 0
                slot_a[b, off:off + k] = slo[mm]

        # one-hot [NB, 128(e), T*128(slot-major)] and its transpose, fp8
        sa = slot_a.reshape(NB, T, 128)
        rng = np.arange(128)
        oh_np = (sa.transpose(0, 2, 1)[:, :, :, None] == rng[None, None, None, :])
        oh_np = np.ascontiguousarray(
            oh_np.reshape(NB, 128, T * 128)).astype(fp8)
        oht_np = (sa[:, None, :, :] == rng[None, :, None, None])
        oht_np = np.ascontiguousarray(
            oht_np.reshape(NB, 128, T * 128)).astype(fp8)

        # per-core x slice in permuted order, chunk-transposed, bf16
        gsel = np.where(perms[c] < NLOC, c * NLOC + perms[c], -1)
        gperm_cores.append(gsel)
        x_own = np.zeros((NPAD, F_IN), np.float32)
        vv = gsel >= 0
        x_own[vv] = x[gsel[vv]]
        x_ownT = np.ascontiguousarray(
            x_own.reshape(NB, 128, F_IN).transpose(0, 2, 1)).astype(
                ml_dtypes.bfloat16)

        dv = np.zeros(NPAD, np.float32)
        sel_src = np.where(perms[c] < NLOC, c * NLOC + perms[c], 0)
        dv[vv] = dinv[sel_src[vv]]
        dinvn_sb = np.ascontiguousarray(dv.reshape(NB, 128).T)

        lane = np.ascontiguousarray(
            slot_a.reshape(NB * T, 128).T).astype(ml_dtypes.bfloat16)
        in_maps.append(dict(
            x_ownT=x_ownT,
            dinvn=dinvn_sb,
            idx_lo=_wrap_idx(idx_lo.reshape(-1)),
            idx_hi=_wrap_idx(idx_hi.reshape(-1)),
            eslot=lane,
            ohm=oh_np,
            oht=oht_np,
        ))

    gperm = np.concatenate(gperm_cores)          # [G] global node id or -1
    return in_maps, gperm, T_LO, T_HI


# ----------------------------------------------------------------------------
# Kernel build
# ----------------------------------------------------------------------------

def build_kernel(T_LO, T_HI, weights):
    T = T_LO + T_HI
    HI0 = LO_LIM if G > LO_LIM else 0  # hi-gather rebase offset
    ndev = 1 if TIMING_1CORE else W
    nc = bacc.Bacc("TRN2", target_bir_lowering=False, debug=False,
                   num_devices=ndev)

    # inputs
    x_ownT = nc.dram_tensor("x_ownT", [NB, F_IN, 128], BF16, kind="ExternalInput")
    dinvn_d = nc.dram_tensor("dinvn", [128, NB], F32, kind="ExternalInput")
    idx_lo_d = nc.dram_tensor("idx_lo", [128, NB * T_LO * 8], I16, kind="ExternalInput")
    idx_hi_d = nc.dram_tensor("idx_hi", [128, NB * T_HI * 8], I16, kind="ExternalInput")
    ohm_d = nc.dram_tensor("ohm", [NB, 128, T * 128], FP8, kind="ExternalInput")
    oht_d = nc.dram_tensor("oht", [NB, 128, T * 128], FP8, kind="ExternalInput")
    eslot_d = nc.dram_tensor("eslot", [128, NB * T], BF16, kind="ExternalInput")
    wnames = ["Wall", "ball", "ident", "iotab"]
    bf16_w = {"Wall", "iotab"}
    w1names = ["Wg0", "Wg1", "bgcn"]
    bf16_w1 = {"Wg0", "Wg1"}
    wshapes = {
        "Wg0": [128, H], "Wg1": [128, H], "bgcn": [128, H],
        "Wall": [H, 512], "ball": [128, 512],
        "ident": [128, 128], "iotab": [128, 128],
    }
    wd = {k: nc.dram_tensor(k, wshapes[k],
                            BF16 if (k in bf16_w or k in bf16_w1) else F32,
                            kind="ExternalInput")
          for k in wnames + w1names}

    out_mu = nc.dram_tensor("out_mu", [NPAD, D], BF16, kind="ExternalOutput")
    out_ls = nc.dram_tensor("out_ls", [NPAD, D], BF16, kind="ExternalOutput")

    # internal DRAM
    hw_bounce = nc.dram_tensor("hw_bounce", [NPAD, H], BF16)
    hw_table = nc.dram_tensor("hw_table", [G, H], BF16, addr_space="Shared")
    kv_bounce = nc.dram_tensor("kv_bounce", [NPAD, 256], BF16)
    kv_table = nc.dram_tensor("kv_table", [G, 256], BF16, addr_space="Shared")

    with tile.TileContext(nc) as tc:
        with (
            tc.tile_pool(name="const", bufs=1) as cp,
            tc.tile_pool(name="persist", bufs=1) as pp,
        ):
            # cross-phase constants
            wt = {}
            for k in wnames:
                t = cp.tile(wshapes[k], BF16 if k in bf16_w else F32, tag=k)
                nc.sync.dma_start(t[:], wd[k][:])
                wt[k] = t
            il = cp.tile([128, NB * T_LO * 8], I16, tag="il")
            nc.sync.dma_start(il[:], idx_lo_d[:])
            ih = cp.tile([128, NB * T_HI * 8], I16, tag="ih")
            nc.sync.dma_start(ih[:], idx_hi_d[:])
            eslot = cp.tile([128, NB * T], BF16, tag="eslot")
            nc.sync.dma_start(eslot[:], eslot_d[:])


            skip_sb = pp.tile([128, NB, 128], BF16, tag="skip")
            qst = pp.tile([128, NB, 128], BF16, tag="qst")

            # ---- Phase 1: GCN ------------------------------------------
            h2tm = tc.tile_pool(name="h2t_pool", bufs=1)
            hp = h2tm.__enter__()
            h2t_sb = hp.tile([128, NPAD], BF16, tag="h2t")
            kvst = hp.tile([128, NB, 256], BF16, tag="kvst")
            p1cm = tc.tile_pool(name="p1const", bufs=1)
            p1cp = p1cm.__enter__()
            for k in w1names:
                t = p1cp.tile(wshapes[k], BF16 if k in bf16_w1 else F32, tag=k)
                nc.sync.dma_start(t[:], wd[k][:])
                wt[k] = t
            dinvn = p1cp.tile([128, NB], F32, tag="dinvn")
            nc.sync.dma_start(dinvn[:], dinvn_d[:])
            hwo = p1cp.tile([128, NB, H], BF16, tag="hwo")

            # ------- Phase 1a: own h' slice, then AllGather the table ----
            with (
                tc.tile_pool(name="p1a_in", bufs=5) as pin,
                tc.tile_pool(name="p1a_ps", bufs=2, space="PSUM") as pps,
            ):
              for _rep in range(REP.get('1a', REPEAT)):
                for g in range(-(-NB // G1B)):
                    b0 = g * G1B
                    nbg = min(G1B, NB - b0)
                    xt = pin.tile([128, nbg, 2, 128], BF16, tag="xt")
                    nc.scalar.dma_start(
                        xt[:], x_ownT[b0:b0 + nbg]
                        .rearrange("g (j p) m -> p g j m", p=128))
                    for j in range(nbg):
                        b = b0 + j
                        ps = pps.tile([128, H], F32, tag="ps")
                        nc.tensor.matmul(ps[:], xt[:, j, 0, :], wt["Wg0"][:],
                                         start=True, stop=False)
                        nc.tensor.matmul(ps[:], xt[:, j, 1, :], wt["Wg1"][:],
                                         start=False, stop=True)
                        # h' = (x @ Wg) * dinv  (norm factored per-node)
                        nc.scalar.mul(hwo[:, b, :], ps[:], dinvn[:, b:b + 1])
                    nc.sync.dma_start(
                        hw_bounce[b0 * 128:(b0 + nbg) * 128, :]
                        .rearrange("(b p) m -> p b m", p=128),
                        hwo[:, b0:b0 + nbg, :])
                    if TIMING_1CORE:
                        nc.sync.dma_start(
                            hw_table[b0 * 128:(b0 + nbg) * 128, :],
                            hw_bounce[b0 * 128:(b0 + nbg) * 128, :])
            if not TIMING_1CORE:
                nc.gpsimd.collective_compute(
                    "AllGather",
                    mybir.AluOpType.bypass,
                    ins=[hw_bounce.ap().opt()],
                    outs=[hw_table.ap().opt()],
                    replica_groups=[list(range(W))],
                )

            # ------- Phase 1b: GCN aggregation + fused kv/q/skip -----------
            NG1 = -(-NB // G1B)
            with (
                tc.tile_pool(name="p1b_g", bufs=3) as pg,
                tc.tile_pool(name="p1b_oh", bufs=4) as poh,
                tc.tile_pool(name="p1b_ps", bufs=3, space="PSUM") as pps1,
                tc.tile_pool(name="p1b_pst", bufs=2, space="PSUM") as ppst,
                tc.tile_pool(name="p1b_pall", bufs=2, space="PSUM") as ppall,
                tc.tile_pool(name="p1b_h2", bufs=4) as ph2,
            ):
              for _rep in range(REP.get('1b', REPEAT)):
                for g in range(NG1):
                    b0 = g * G1B
                    nbg = min(G1B, NB - b0)
                    gl = pg.tile([128, nbg * T_LO, H], BF16, tag="gl")
                    SPL = 3
                    for c0 in range(0, nbg, SPL):
                        cn = min(SPL, nbg - c0)
                        nc.gpsimd.dma_gather(
                            gl[:, c0 * T_LO:(c0 + cn) * T_LO, :],
                            hw_table[:],
                            il[:, (b0 + c0) * T_LO * 8:
                               (b0 + c0 + cn) * T_LO * 8],
                            cn * T_LO * 128, cn * T_LO * 128, H,
                            single_packet=False)
                    gh = pg.tile([128, nbg * T_HI, H], BF16, tag="gh")
                    for c0 in range(0, nbg, SPL):
                        cn = min(SPL, nbg - c0)
                        nc.gpsimd.dma_gather(
                            gh[:, c0 * T_HI:(c0 + cn) * T_HI, :],
                            hw_table[HI0:, :],
                            ih[:, (b0 + c0) * T_HI * 8:
                               (b0 + c0 + cn) * T_HI * 8],
                            cn * T_HI * 128, cn * T_HI * 128, H,
                            single_packet=False)
                    for j in range(nbg):
                        b = b0 + j
                        oh1 = poh.tile([128, T, 128], BF16, tag="oh1")
                        esl = eslot[:, b * T:(b + 1) * T]
                        nc.vector.tensor_tensor(
                            oh1[:],
                            esl.rearrange("p (c one) -> p c one", one=1)
                            .broadcast_to([128, T, 128]),
                            wt["iotab"][:]
                            .rearrange("p (one s) -> p one s", one=1)
                            .broadcast_to([128, T, 128]),
                            op=mybir.AluOpType.is_equal)
                        ps1 = pps1.tile([128, H], F32, tag="ps1")
                        for t in range(T_LO):
                            nc.tensor.matmul(ps1[:], oh1[:, t, :],
                                             gl[:, j * T_LO + t, :],
                                             start=(t == 0), stop=False)
                        for t in range(T_HI):
                            nc.tensor.matmul(ps1[:], oh1[:, T_LO + t, :],
                                             gh[:, j * T_HI + t, :],
                                             start=False, stop=(t == T_HI - 1))
                        # h2 = leaky((agg + h'_own) * dinv + bias)
                        h2c = ph2.tile([128, H], F32, tag="h2c")
                        nc.vector.tensor_tensor(h2c[:], ps1[:], hwo[:, b, :],
                                                op=mybir.AluOpType.add)
                        nc.vector.scalar_tensor_tensor(
                            h2c[:], h2c[:], dinvn[:, b:b + 1], wt["bgcn"][:],
                            op0=mybir.AluOpType.mult, op1=mybir.AluOpType.add)
                        nc.vector.scalar_tensor_tensor(
                            h2c[:], h2c[:], LEAKY, h2c[:],
                            op0=mybir.AluOpType.mult, op1=mybir.AluOpType.max)
                        pst = ppst.tile([128, 128], F32, tag="pst")
                        nc.tensor.transpose(pst[:], h2c[:], wt["ident"][:])
                        hsl = h2t_sb[:, b * 128:(b + 1) * 128]
                        nc.scalar.copy(hsl, pst[:])
                        # fused [kv | q | skip] projection of the fresh
                        # chunk; biases preloaded into PSUM via Act
                        pall = ppall.tile([128, 512], F32, tag="pall")
                        nc.scalar.copy(pall[:], wt["ball"][:])
                        nc.tensor.matmul(pall[:], hsl, wt["Wall"][:],
                                         start=False, stop=True,
                                         skip_group_check=True)
                        nc.scalar.copy(kvst[:, b, :], pall[:, 0:256])
                        nc.scalar.copy(qst[:, b, :], pall[:, 256:384])
                        nc.scalar.copy(skip_sb[:, b, :], pall[:, 384:512])
                    nc.sync.dma_start(
                        kv_bounce[b0 * 128:(b0 + nbg) * 128, :]
                        .rearrange("(b p) m -> p b m", p=128),
                        kvst[:, b0:b0 + nbg, :])
                    if TIMING_1CORE:
                        nc.sync.dma_start(
                            kv_table[b0 * 128:(b0 + nbg) * 128, :],
                            kv_bounce[b0 * 128:(b0 + nbg) * 128, :])

            p1cm.__exit__(None, None, None)
            h2tm.__exit__(None, None, None)
            if not TIMING_1CORE:
                nc.gpsimd.collective_compute(
                    "AllGather",
                    mybir.AluOpType.bypass,
                    ins=[kv_bounce.ap().opt()],
                    outs=[kv_table.ap().opt()],
                    replica_groups=[list(range(W))],
                )

            # ---------------- Phase 2b: transformer aggregation ------------
            NG2 = -(-NB // G2B)
            with (
                tc.tile_pool(name="p2b_kv", bufs=4) as pkv,
                tc.tile_pool(name="p2b_oh", bufs=2) as poh2,
                tc.tile_pool(name="p2b_qe", bufs=2) as pqe,
                tc.tile_pool(name="p2b_qeps", bufs=1, space="PSUM") as pqeps,
                tc.tile_pool(name="p2b_pay", bufs=3) as ppay,
                tc.tile_pool(name="p2b_ex", bufs=6) as pex,
                tc.tile_pool(name="p2b_ps", bufs=2, space="PSUM") as pps3,
                tc.tile_pool(name="p2b_out", bufs=1) as pob,
            ):
              omst = pob.tile([128, NB, D], BF16, tag="omst")
              olst = pob.tile([128, NB, D], BF16, tag="olst")
              for _rep in range(REP.get('2b', REPEAT)):
                for g in range(NG2):
                    b0 = g * G2B
                    nbg = min(G2B, NB - b0)
                    kl = pkv.tile([128, nbg * T_LO, 256], BF16, tag="kl")
                    for c0 in range(nbg):
                        nc.gpsimd.dma_gather(
                            kl[:, c0 * T_LO:(c0 + 1) * T_LO, :],
                            kv_table[:],
                            il[:, (b0 + c0) * T_LO * 8:
                               (b0 + c0 + 1) * T_LO * 8],
                            T_LO * 128, T_LO * 128, 256,
                            single_packet=False)
                    kh = pkv.tile([128, nbg * T_HI, 256], BF16, tag="kh")
                    for c0 in range(nbg):
                        nc.gpsimd.dma_gather(
                            kh[:, c0 * T_HI:(c0 + 1) * T_HI, :],
                            kv_table[HI0:, :],
                            ih[:, (b0 + c0) * T_HI * 8:
                               (b0 + c0 + 1) * T_HI * 8],
                            T_HI * 128, T_HI * 128, 256,
                            single_packet=False)
                    oh = poh2.tile([128, nbg, T, 128], FP8, tag="oh2")
                    nc.sync.dma_start(
                        oh[:],
                        ohm_d[b0:b0 + nbg].rearrange("g p f -> p g f")
                        .rearrange("p g (t s) -> p g t s", s=128))
                    oht = poh2.tile([128, nbg, T, 128], FP8, tag="oht")
                    nc.sync.dma_start(
                        oht[:],
                        oht_d[b0:b0 + nbg].rearrange("g p f -> p g f")
                        .rearrange("p g (t s) -> p g t s", s=128))

                    for j in range(nbg):
                        b = b0 + j
                        qeps = pqeps.tile([128, T, 128], F32, tag="qeps")
                        for t in range(T):
                            nc.tensor.matmul(qeps[:, t, :], oht[:, j, t, :],
                                             qst[:, b, :],
                                             start=True, stop=True)
                        qe = pqe.tile([128, T, 128], BF16, tag="qe")
                        nc.scalar.copy(qe[:], qeps[:])

                        al = pex.tile([128, T, 2], F32, tag="al")
                        tm = ppay.tile([128, T, 128], FP16, tag="tm")
                        nc.vector.tensor_tensor(
                            tm[:, 0:T_LO, :], qe[:, 0:T_LO, :],
                            kl[:, j * T_LO:(j + 1) * T_LO, 0:128],
                            op=mybir.AluOpType.mult)
                        nc.vector.tensor_tensor(
                            tm[:, T_LO:T, :], qe[:, T_LO:T, :],
                            kh[:, j * T_HI:(j + 1) * T_HI, 0:128],
                            op=mybir.AluOpType.mult)
                        tmv = tm[:].rearrange("p t (c f) -> p t c f", c=2)
                        t2 = pex.tile([128, T, 2, 32], FP16, tag="t2")
                        nc.vector.tensor_tensor(
                            t2[:], tmv[:, :, :, 0:32], tmv[:, :, :, 32:64],
                            op=mybir.AluOpType.add)
                        t3 = pex.tile([128, T, 2, 16], FP16, tag="t3")
                        nc.vector.tensor_tensor(
                            t3[:], t2[:, :, :, 0:16], t2[:, :, :, 16:32],
                            op=mybir.AluOpType.add)
                        t4 = pex.tile([128, T, 2, 8], FP16, tag="t4")
                        nc.vector.tensor_tensor(
                            t4[:], t3[:, :, :, 0:8], t3[:, :, :, 8:16],
                            op=mybir.AluOpType.add)
                        nc.vector.tensor_reduce(
                            al[:], t4[:],
                            axis=mybir.AxisListType.X, op=mybir.AluOpType.add)
                        ex = pex.tile([128, T, 2], BF16, tag="ex")
                        nc.scalar.activation(ex[:], al[:],
                                             mybir.ActivationFunctionType.Exp,
                                             scale=0.125)

                        pay = ppay.tile([128, T, 130], BF16, tag="pay")
                        nc.vector.tensor_tensor(
                            pay[:, 0:T_LO, 0:128]
                            .rearrange("p t (f c) -> p t f c", c=2),
                            kl[:, j * T_LO:(j + 1) * T_LO, 128:256]
                            .rearrange("p t (f c) -> p t f c", c=2),
                            ex[:, 0:T_LO, :]
                            .rearrange("p t (one c) -> p t one c", one=1)
                            .broadcast_to([128, T_LO, 64, 2]),
                            op=mybir.AluOpType.mult)
                        nc.vector.tensor_tensor(
                            pay[:, T_LO:T, 0:128]
                            .rearrange("p t (f c) -> p t f c", c=2),
                            kh[:, j * T_HI:(j + 1) * T_HI, 128:256]
                            .rearrange("p t (f c) -> p t f c", c=2),
                            ex[:, T_LO:T, :]
                            .rearrange("p t (one c) -> p t one c", one=1)
                            .broadcast_to([128, T_HI, 64, 2]),
                            op=mybir.AluOpType.mult)
                        nc.scalar.copy(pay[:, :, 128:130], ex[:])

                        ps = pps3.tile([128, 130], F32, tag="ps2b")
                        for t in range(T):
                            nc.tensor.matmul(ps[:], oh[:, j, t, :],
                                             pay[:, t, :],
                                             start=(t == 0), stop=(t == T - 1))

                        den = pex.tile([128, 2], F32, tag="den")
                        nc.vector.tensor_scalar_add(den[:], ps[:, 128:130],
                                                    1e-16)
                        nc.vector.reciprocal(den[:], den[:])
                        psv = ps[:, 0:128].rearrange("p (f c) -> p f c", c=2)
                        nc.vector.scalar_tensor_tensor(
                            omst[:, b, :].rearrange("p (f one) -> p f one",
                                                    one=1),
                            psv[:, :, 0:1], den[:, 0:1],
                            skip_sb[:, b, 0:64]
                            .rearrange("p (f one) -> p f one", one=1),
                            op0=mybir.AluOpType.mult, op1=mybir.AluOpType.add)
                        nc.vector.scalar_tensor_tensor(
                            olst[:, b, :].rearrange("p (f one) -> p f one",
                                                    one=1),
                            psv[:, :, 1:2], den[:, 1:2],
                            skip_sb[:, b, 64:128]
                            .rearrange("p (f one) -> p f one", one=1),
                            op0=mybir.AluOpType.mult, op1=mybir.AluOpType.add)
                        nc.vector.tensor_scalar_min(olst[:, b, :],
                                                    olst[:, b, :], MAX_LOGSTD)
                    nc.sync.dma_start(
                        out_mu[b0 * 128:(b0 + nbg) * 128, :]
                        .rearrange("(b p) m -> p b m", p=128),
                        omst[:, b0:b0 + nbg, :])
                    nc.scalar.dma_start(
                        out_ls[b0 * 128:(b0 + nbg) * 128, :]
                        .rearrange("(b p) m -> p b m", p=128),
                        olst[:, b0:b0 + nbg, :])

    nc.compile()
    return nc


def make_weight_inputs(W_gcn, b_gcn, Wq_mu, bq_mu, Wk_mu, bk_mu, Wv_mu, bv_mu,
                       Ws_mu, bs_mu, Wq_ls, bq_ls, Wk_ls, bk_ls, Wv_ls, bv_ls,
                       Ws_ls, bs_ls):
    import ml_dtypes
    f = np.float32
    bf = ml_dtypes.bfloat16
    rep = lambda v: np.tile(np.asarray(v, f)[None, :], (128, 1))
    return dict(
        Wg0=np.ascontiguousarray(W_gcn[:128]).astype(bf),
        Wg1=np.ascontiguousarray(W_gcn[128:]).astype(bf),
        bgcn=rep(b_gcn),
        Wall=np.asarray(np.hstack(
            [Wk_mu, Wk_ls,
             np.stack([Wv_mu, Wv_ls], axis=2).reshape(Wv_mu.shape[0], -1),
             Wq_mu, Wq_ls, Ws_mu, Ws_ls]), bf),
        ball=rep(np.hstack([np.zeros(256, np.float32), bq_mu, bq_ls,
                            bs_mu + bv_mu, bs_ls + bv_ls])),
        ident=np.eye(128, dtype=f),
        iotab=np.tile(np.arange(128, dtype=f), (128, 1)).astype(bf),
    )


_CACHE = {}
PROFILE = False
LAST_EXEC_NS = None
REPEAT = 1      # re-run compute phases (device-time slope measurement)
REP = {}        # per-phase repeat override: {'1a':k,'1b':k,'2a':k,'2b':k}
TIMING_1CORE = False  # build single-core variant (collectives -> local copies)


def kernel(x, edge_index, **weights):
    in_maps, gperm, T_LO, T_HI = preprocess(x, edge_index)
    wmap = make_weight_inputs(**weights)
    for m in in_maps:
        m.update(wmap)

    key = (T_LO, T_HI)
    if key not in _CACHE:
        _CACHE[key] = build_kernel(T_LO, T_HI, weights)
    nc = _CACHE[key]

    global LAST_EXEC_NS
    res = run_bass_kernel_spmd(nc, in_maps, core_ids=list(range(W)),
                               trace=PROFILE)
    LAST_EXEC_NS = res.exec_time_ns

    mu = np.empty((N, D), np.float32)
    ls = np.empty((N, D), np.float32)
    for c in range(W):
        om = np.asarray(res.results[c]["out_mu"], np.float32)
        ol = np.asarray(res.results[c]["out_ls"], np.float32)
        gsel = gperm[c * NPAD:(c + 1) * NPAD]
        v = gsel >= 0
        mu[gsel[v]] = om[v]
        ls[gsel[v]] = ol[v]

    # bv is folded into the skip bias on-device (valid because softmax
    # weights sum to 1); nodes with zero in-degree have no attention term,
    # so undo the fold for them.
    dst = np.asarray(edge_index[1])
    indeg = np.bincount(dst, minlength=N)
    zd = np.nonzero(indeg == 0)[0]
    if zd.size:
        mu[zd] -= np.asarray(weights["bv_mu"], np.float32)
        ls[zd] = np.minimum(ls[zd] - np.asarray(weights["bv_ls"], np.float32),
                            MAX_LOGSTD)
    return mu, ls
